# revision 32
# baseline (speedup 1.0000x reference)
"""Trainium2 Bass kernel for nn_DOSAConLoss (density/hardness-weighted focal CIoU
+ focal BCE + O(N^2) pairwise contrastive hinge loss).

Strategy (8 NeuronCores, shard N=8192 rows -> 1024 rows/core):
  * loss_loc  : per-row CIoU pipeline on each core's row shard (DVE+ACT, fp32).
  * loss_cls  : per-row focal BCE pipeline on each core's row shard (fp32).
  * contrast  : the hinge term max(1-dist,0)^2 is nonzero only for pairs with
                squared distance d2 < 1.  Each core screens its 1024 rows
                against ALL 8192 columns via a PE matmul producing
                2*e_i.e_j in PSUM (bf16 inputs).  The self-pair (d2=0) is
                killed on the PE itself by accumulating -BIG*I onto the
                diagonal block (each core's e^T copy is rolled by its row
                offset so the diagonal sits at a core-invariant position and
                the SPMD program can be shared).  Column groups of 2048 are
                then screened by one of two fused single-instruction paths:
                  - DVE: reduce_max -> per-row group maximum, host compares
                    against  sq_i + min_group(sq_j) - MARGIN.
                  - ACT: Relu(2dot + (MARGIN - sq_i - min_group(sq_j)))
                    with accum_out -> a sum certificate that is 0 iff no
                    pair in the group can have d2 <= MARGIN.
                If every group certifies (true for any plausible input:
                random 128-dim embeddings have min pairwise d2 ~ 92, and the
                min certified bound on this data is ~37 >> MARGIN=16 >> bf16
                error ~2), the contrastive sum is exactly 0.  Otherwise the
                host falls back to an exact numpy evaluation of the term.

Inputs are the FULL tensors from setup_inputs(); output is the scalar loss.
"""

import os
import sys

for _p in ("/opt/trn_rl_repo", "/root/.axon_site/_ro/trn_rl_repo"):
    if os.path.isdir(_p) and _p not in sys.path:
        sys.path.insert(0, _p)

from contextlib import ExitStack

import ml_dtypes
import numpy as np

import concourse.bacc as bacc
import concourse.bass as bass
import concourse.tile as tile
from concourse.tile_rust import add_dep_helper
from concourse import mybir
from concourse.bass_utils import run_bass_kernel_spmd

F32 = mybir.dt.float32
F16 = mybir.dt.float16
BF16 = mybir.dt.bfloat16
BF16_NP = ml_dtypes.bfloat16
ALU = mybir.AluOpType
AF = mybir.ActivationFunctionType

N, D, C = 8192, 128, 80
NCORES = 8
RPC = N // NCORES          # rows per core = 1024
NRB = RPC // 128           # row blocks per core = 8
CHUNK = 512                # one PSUM bank of fp32
GRP = 2048                 # column group = 4 banks
MARGIN = 16.0              # certificate slack (bf16 dot error is < ~2)
BIG = 1.0e30

# Triangular screening with per-row-block sliding windows: in a core's rolled
# frame (local col l = global j - core_base mod N), row-block rb screens
# l in [rb*128, rb*128 + WIN).  A pair at forward distance d is covered by the
# i-side block if d <= WIN-128, or by the j-side block if N-d <= WIN-128;
# min(d, N-d) <= N/2 so WIN = N/2 + 128 = 4224 covers every unordered pair.
WIN = N // 2 + 128
SCOLS = RPC - 128 + WIN    # = 5120 columns of rolled e^T actually touched
UNITS = ((0, 2048), (2048, 2048), (4096, 128))   # (win offset, width)
NU = len(UNITS)
# units flipped from the default parity assignment to balance DVE vs ACT
_FLIP = {(0, 0), (4, 0)}
N_WARM = 32                # PE warm-up matmuls during the DMA ramp


def group_kind(rb, u):
    """Static engine assignment for a (row-block, column-unit) tile —
    alternating so the DVE max-reduce and ACT sum-certificate paths overlap."""
    kind = "act" if (rb * NU + u) % 2 == 1 else "dve"
    if (rb, u) in _FLIP:
        kind = "act" if kind == "dve" else "dve"
    return kind

GAMMA_LOCAL = 2.5
ALPHA = 1.2
DELTA = 1.0
TAU = 0.3
LAMBDA_CONTRAST = 0.5
EPS = 1e-7


# --------------------------------------------------------------------------
# device program
# --------------------------------------------------------------------------

def build_program():
    nc = bacc.Bacc("TRN2", target_bir_lowering=False, debug=False,
                   num_devices=NCORES)

    # packed inputs: one bf16 blob (idmats | lhsT2 | eT) and one f32 blob
    # (clsx | clst | boxp | boxt | dens | abias) to minimize DMA count
    BFW = 256 + RPC + SCOLS
    FW = NRB * C * 2 + 32 + 32 + NRB + NRB * NU
    inbf = nc.dram_tensor("inbf", [128, BFW], BF16, kind="ExternalInput")
    inf32 = nc.dram_tensor("inf32", [128, FW], F32, kind="ExternalInput")

    redout = nc.dram_tensor("redout", [128, 2 * NRB * NU + 1], F32,
                            kind="ExternalOutput")
    opart = nc.dram_tensor("opart", [128, 2], F32, kind="ExternalOutput")

    with tile.TileContext(nc) as tc:
        with ExitStack() as ctx:
            consts = ctx.enter_context(tc.tile_pool(name="consts", bufs=1))
            psums = ctx.enter_context(
                tc.tile_pool(name="psums", bufs=2, space="PSUM"))
            scr = ctx.enter_context(tc.tile_pool(name="scr", bufs=3))
            work = ctx.enter_context(tc.tile_pool(name="work", bufs=1))

            bfb = consts.tile([128, BFW], BF16)
            f32b = consts.tile([128, FW], F32)
            nc.sync.dma_start(out=f32b[:], in_=inf32.ap())
            nc.sync.dma_start(out=bfb[:, :256 + RPC],
                              in_=inbf.ap()[:, :256 + RPC])
            for c0 in range(256 + RPC, BFW, 2048):
                w = min(2048, BFW - c0)
                nc.sync.dma_start(out=bfb[:, c0:c0 + w],
                                  in_=inbf.ap()[:, c0:c0 + w])
            id_s = bfb[:, 0:256]
            lhsT_s = bfb[:, 256:256 + RPC]
            eT_s = bfb[:, 256 + RPC:]
            FC_ = NRB * C
            clsx_v = f32b[:, 0:FC_]
            clst_v = f32b[:, FC_:2 * FC_]
            boxp_v = f32b[:, 2 * FC_:2 * FC_ + 32]
            boxt_v = f32b[:, 2 * FC_ + 32:2 * FC_ + 64]
            dens_v = f32b[:, 2 * FC_ + 64:2 * FC_ + 64 + NRB]
            abias_s = f32b[:, 2 * FC_ + 64 + NRB:]

            NRED = NRB * NU
            red = consts.tile([128, 2 * NRED + 1], F32)
            nc.vector.memset(red[:], -1.0)
            part = consts.tile([128, 2], F32)
            bias0 = consts.tile([128, 1], F32)
            nc.vector.memset(bias0[:], 0.0)
            bias25 = consts.tile([128, 1], F32)
            nc.vector.memset(bias25[:], 2.5)
            bias1 = consts.tile([128, 1], F32)
            nc.vector.memset(bias1[:], 1.0)

            # ---------------- focal BCE (cls) — part 1 ----------------
            FC = NRB * C
            x = clsx_v
            t = clst_v

            # softplus(-x) = ln(1 + exp(-x))   [exp/ln share one ACT table set]
            sp = work.tile([128, FC], F32)
            xn = work.tile([128, FC], F32)
            nc.vector.tensor_scalar(out=xn[:], in0=x, scalar1=-1.0,
                                    scalar2=80.0, op0=ALU.mult, op1=ALU.min)
            act_chain = []
            act_chain.append(
                nc.scalar.activation(xn[:], xn[:], AF.Exp, bias=bias0[:]))
            act_chain.append(
                nc.scalar.activation(sp[:], xn[:], AF.Ln, bias=bias1[:]))
            pr = work.tile([128, FC], F32)   # sigmoid(x)
            act_chain.append(
                nc.scalar.activation(pr[:], x, AF.Sigmoid, bias=bias0[:]))

            tx = work.tile([128, FC], F32)
            nc.vector.tensor_mul(tx[:], t, x)
            bce = work.tile([128, FC], F32)  # sp + x - t*x
            nc.vector.tensor_add(bce[:], sp[:], x)
            nc.vector.tensor_sub(bce[:], bce[:], tx)

            tp = work.tile([128, FC], F32)
            nc.vector.tensor_mul(tp[:], t, pr[:])
            w = work.tile([128, FC], F32)
            nc.vector.tensor_add(w[:], t, pr[:])
            q = work.tile([128, FC], F32)    # 1 - p_t = t + p - 2tp
            nc.vector.scalar_tensor_tensor(
                out=q[:], in0=tp[:], scalar=-2.0, in1=w[:],
                op0=ALU.mult, op1=ALU.add)
            nc.vector.tensor_scalar_max(q[:], q[:], 0.0)

            # ---------------- CIoU localization — part 1 ----------------
            NB = NRB
            bp = boxp_v.rearrange("p (c b) -> p c b", c=4)
            bt = boxt_v.rearrange("p (c b) -> p c b", c=4)
            dn = dens_v

            px, py, pw, ph = (bp[:, i, :] for i in range(4))
            tx_, ty_, tw, th = (bt[:, i, :] for i in range(4))

            loc = ctx.enter_context(tc.tile_pool(name="loc", bufs=1))

            def lt(name):
                return loc.tile([128, NB], F32, name=name)

            hw1, hh1, hw2, hh2 = lt("hw1"), lt("hh1"), lt("hw2"), lt("hh2")
            nc.vector.tensor_scalar_mul(hw1[:], pw, 0.5)
            nc.vector.tensor_scalar_mul(hh1[:], ph, 0.5)
            nc.vector.tensor_scalar_mul(hw2[:], tw, 0.5)
            nc.vector.tensor_scalar_mul(hh2[:], th, 0.5)

            l1, r1, t1, b1 = lt("l1"), lt("r1"), lt("t1"), lt("b1")
            l2, r2, t2, b2 = lt("l2"), lt("r2"), lt("t2"), lt("b2")
            nc.vector.tensor_sub(l1[:], px, hw1[:])
            nc.vector.tensor_add(r1[:], px, hw1[:])
            nc.vector.tensor_sub(t1[:], py, hh1[:])
            nc.vector.tensor_add(b1[:], py, hh1[:])
            nc.vector.tensor_sub(l2[:], tx_, hw2[:])
            nc.vector.tensor_add(r2[:], tx_, hw2[:])
            nc.vector.tensor_sub(t2[:], ty_, hh2[:])
            nc.vector.tensor_add(b2[:], ty_, hh2[:])

            # intersection / union / iou
            ltx, lty, rbx, rby = lt("ltx"), lt("lty"), lt("rbx"), lt("rby")
            nc.vector.tensor_max(ltx, l1[:], l2[:])
            nc.vector.tensor_max(lty[:], t1[:], t2[:])
            nc.vector.tensor_tensor(out=rbx, in0=r1[:], in1=r2[:], op=ALU.min)
            nc.vector.tensor_tensor(out=rby[:], in0=b1[:], in1=b2[:], op=ALU.min)
            iw, ih = lt("iw"), lt("ih")
            nc.vector.tensor_sub(iw[:], rbx, ltx)
            nc.vector.tensor_scalar_max(iw[:], iw[:], 0.0)
            nc.vector.tensor_sub(ih[:], rby[:], lty[:])
            nc.vector.tensor_scalar_max(ih[:], ih[:], 0.0)
            inter = lt("inter")
            nc.vector.tensor_mul(inter[:], iw[:], ih[:])
            area1, area2 = lt("area1"), lt("area2")
            nc.vector.tensor_mul(area1[:], pw, ph)
            nc.vector.tensor_mul(area2[:], tw, th)
            union = lt("union")
            nc.vector.tensor_add(union[:], area1[:], area2[:])
            nc.vector.scalar_tensor_tensor(
                out=union[:], in0=inter[:], scalar=-1.0, in1=union[:],
                op0=ALU.mult, op1=ALU.add)
            nc.vector.tensor_scalar_add(union[:], union[:], EPS)
            iou, runion = lt("iou"), lt("runion")
            nc.vector.reciprocal(runion[:], union[:])
            nc.vector.tensor_mul(iou[:], inter[:], runion[:])

            # enclosing box diagonal^2
            cw, chh, c2 = lt("cw"), lt("chh"), lt("c2")
            tmp = lt("tmp")
            nc.vector.tensor_max(tmp[:], r1[:], r2[:])
            nc.vector.tensor_tensor(out=cw[:], in0=l1[:], in1=l2[:], op=ALU.min)
            nc.vector.tensor_sub(cw[:], tmp[:], cw[:])
            nc.vector.tensor_max(tmp[:], b1[:], b2[:])
            nc.vector.tensor_tensor(out=chh[:], in0=t1[:], in1=t2[:], op=ALU.min)
            nc.vector.tensor_sub(chh[:], tmp[:], chh[:])
            nc.vector.tensor_mul(c2[:], cw[:], cw[:])
            nc.vector.tensor_mul(tmp[:], chh[:], chh[:])
            nc.vector.tensor_add(c2[:], c2[:], tmp[:])
            nc.vector.tensor_scalar_add(c2[:], c2[:], EPS)

            # center distance^2
            dx, dy, rho2 = lt("dx"), lt("dy"), lt("rho2")
            nc.vector.tensor_sub(dx, tx_, px)
            nc.vector.tensor_sub(dy[:], ty_, py)
            nc.vector.tensor_mul(rho2[:], dx, dx)
            nc.vector.tensor_mul(tmp[:], dy[:], dy[:])
            nc.vector.tensor_add(rho2[:], rho2[:], tmp[:])

            # v = 4/pi^2 (atan(w2/h2') - atan(w1/h1'))^2
            rat1, rat2, at1, at2, v = (lt("rat1"), lt("rat2"), lt("at1"),
                                       lt("at2"), lt("v"))
            nc.vector.tensor_scalar_add(tmp[:], ph, EPS)
            nc.vector.reciprocal(tmp[:], tmp[:])
            nc.vector.tensor_mul(rat1[:], pw, tmp[:])
            nc.vector.tensor_scalar_add(tmp[:], th, EPS)
            nc.vector.reciprocal(tmp[:], tmp[:])
            nc.vector.tensor_mul(rat2[:], tw, tmp[:])

            # HW arctan LUT covers [-pi/2, pi/2] only; for r > 1 use
            # arctan(r) = pi/2 - arctan(1/r)  (r > 0 always here).
            rr, rmin, mgt = lt("rr"), lt("rmin"), lt("mgt")
            for rat, at in ((rat1, at1), (rat2, at2)):
                nc.vector.reciprocal(rr[:], rat)
                nc.vector.tensor_tensor(out=rmin[:], in0=rat, in1=rr[:],
                                        op=ALU.min)
                act_chain.append(nc.scalar.activation(
                    at, rmin[:], AF.Arctan, bias=bias0[:]))
                nc.vector.tensor_scalar(out=mgt, in0=rat, scalar1=1.0,
                                        scalar2=None, op0=ALU.is_gt)
                # at + m*(pi/2 - 2*at)
                nc.vector.tensor_scalar(out=rr[:], in0=at, scalar1=-2.0,
                                        scalar2=float(np.pi / 2),
                                        op0=ALU.mult, op1=ALU.add)
                nc.vector.tensor_mul(mgt, mgt, rr[:])
                nc.vector.tensor_add(at, at, mgt)
            nc.vector.tensor_sub(v[:], at2[:], at1[:])
            nc.vector.tensor_mul(v[:], v[:], v[:])
            nc.vector.tensor_scalar_mul(v[:], v[:], 4.0 / (np.pi ** 2))

            # alpha = v / (v - iou + 1 + eps)
            den, alpha = lt("den"), lt("alpha")
            nc.vector.scalar_tensor_tensor(
                out=den[:], in0=iou[:], scalar=-1.0, in1=v[:],
                op0=ALU.mult, op1=ALU.add)
            nc.vector.tensor_scalar_add(den[:], den[:], 1.0 + EPS)
            nc.vector.reciprocal(den[:], den[:])
            nc.vector.tensor_mul(alpha[:], v[:], den[:])

            # ciou = iou - (rho2/c2 + v*alpha)
            ciou = lt("ciou")
            nc.vector.reciprocal(tmp[:], c2[:])
            nc.vector.tensor_mul(tmp[:], rho2[:], tmp[:])
            nc.vector.tensor_mul(alpha[:], v[:], alpha[:])
            nc.vector.tensor_add(tmp[:], tmp[:], alpha[:])
            nc.vector.tensor_sub(ciou[:], iou[:], tmp[:])

            # hardness weight sigmoid(2.5 - 5*ciou)  [sigmoid table set
            # still loaded from the arctan/sigmoid group]
            dwt, hwt = lt("dwt"), lt("hwt")
            act_chain.append(
                nc.scalar.activation(hwt, ciou[:], AF.Sigmoid,
                                     scale=-5.0, bias=bias25[:]))

            # ---------------- focal BCE (cls) — part 2 (sqrt set) ----------------
            rootq = work.tile([128, FC], F32)
            act_chain.append(
                nc.scalar.activation(rootq[:], q[:], AF.Sqrt, bias=bias0[:]))
            mod = work.tile([128, FC], F32)  # q^1.5
            nc.vector.tensor_mul(mod[:], q[:], rootq[:])
            af = work.tile([128, FC], F32)   # 0.75 - 0.5*t
            nc.vector.tensor_scalar(
                out=af[:], in0=t, scalar1=-0.5, scalar2=0.75,
                op0=ALU.mult, op1=ALU.add)
            prod = work.tile([128, FC], F32)
            nc.vector.tensor_mul(prod[:], bce[:], mod[:])
            el = work.tile([128, FC], F32)
            nc.vector.tensor_mul(el[:], prod[:], af[:])
            nc.vector.reduce_sum(out=part[:, 1:2], in_=el[:],
                                 axis=mybir.AxisListType.X)

            # ---------------- CIoU localization — part 2 ----------------
            onem, p25 = lt("onem"), lt("p25")
            nc.vector.tensor_scalar(
                out=onem[:], in0=ciou[:], scalar1=-1.0, scalar2=1.0,
                op0=ALU.mult, op1=ALU.add)
            nc.vector.tensor_scalar_max(onem[:], onem[:], 0.0)
            nc.vector.tensor_mul(p25[:], onem[:], onem[:])
            act_chain.append(
                nc.scalar.activation(tmp[:], onem[:], AF.Sqrt, bias=bias0[:]))
            nc.vector.tensor_mul(p25[:], p25[:], tmp[:])   # (1-ciou)^2.5

            saf = lt("saf")
            nc.vector.tensor_scalar_add(tmp[:], area2[:], 1e-7)
            nc.vector.reciprocal(tmp[:], tmp[:])
            nc.vector.tensor_mul(saf[:], p25[:], tmp[:])

            nc.vector.tensor_scalar(
                out=dwt[:], in0=dn, scalar1=ALPHA, scalar2=1.0,
                op0=ALU.mult, op1=ALU.add)
            nc.vector.tensor_mul(dwt[:], dwt[:], hwt[:])
            locel = lt("locel")
            nc.vector.tensor_mul(locel[:], dwt[:], saf[:])
            nc.vector.reduce_sum(out=part[:, 0:1], in_=locel[:],
                                 axis=mybir.AxisListType.X)

            # PE warm-up: ~3.5us of dummy matmuls during the input DMA so the
            # HAM clock gate releases before the real stream starts.  The
            # result is reduced into a scrap column so DCE keeps it.
            wpt = psums.tile([128, GRP], F32, name="wpt", tag="pt")
            for i in range(N_WARM):
                nc.tensor.matmul(wpt[:, 0:128], id_s[:, 0:128],
                                 id_s[:, 0:128], start=(i == 0),
                                 stop=(i == N_WARM - 1))
            nc.vector.reduce_max(out=red[:, 2 * NRED:2 * NRED + 1],
                                 in_=wpt[:, 0:128], axis=mybir.AxisListType.X)

            # ------------- pairwise screen: max(2*dot) / cert sums -------------
            for rb in range(NRB):
                lhs_slice = lhsT_s[:, rb * 128:(rb + 1) * 128]
                base = rb * 128            # window start; diag block at offset 0
                for u, (c0, w) in enumerate(UNITS):
                    pt = psums.tile([128, GRP], F32, name="pt", tag="pt")
                    for cc in range((w + CHUNK - 1) // CHUNK):
                        cw = min(CHUNK, w - cc * CHUNK)
                        nc.tensor.matmul(
                            pt[:, cc * CHUNK:cc * CHUNK + cw], lhs_slice,
                            eT_s[:, base + c0 + cc * CHUNK:
                                 base + c0 + cc * CHUNK + cw],
                            start=True, stop=not (u == 0 and cc == 0))
                    if u == 0:
                        # kill the self-pair: accumulate -BIG*I onto the
                        # diagonal 128x128 block (window offset 0)
                        nc.tensor.matmul(
                            pt[:, 0:128],
                            id_s[:, 128:256], id_s[:, 0:128],
                            start=False, stop=True)
                    col = rb * NU + u
                    if group_kind(rb, u) == "dve":
                        nc.vector.reduce_max(
                            out=red[:, col:col + 1], in_=pt[:, :w],
                            axis=mybir.AxisListType.X)
                    else:
                        o16 = scr.tile([128, GRP], F16, name="o16",
                                       tag="act_scratch")
                        nc.scalar.activation(
                            o16[:, :w], pt[:, :w], AF.Relu,
                            bias=abias_s[:, col:col + 1],
                            scale=1.0,
                            accum_out=red[:, NRED + col:NRED + col + 1])

            nc.sync.dma_start(out=redout.ap(), in_=red[:])

            nc.sync.dma_start(out=opart.ap(), in_=part[:])

            # pin the transcendental order so the ACT table sets load at most
            # once each: [exp ln] [sigmoid arctan arctan sigmoid] [sqrt sqrt]
            for a, b in zip(act_chain[1:], act_chain[:-1]):
                add_dep_helper(a.ins, b.ins, sync=False,
                               reason="group ACT calls by table set")

    nc.compile()
    return nc


# --------------------------------------------------------------------------
# host-side prep / gather
# --------------------------------------------------------------------------

def _prep_in_maps(pred_boxes, pred_cls, target_boxes, target_cls,
                  embeddings, density_map):
    idmats = np.zeros((128, 256), BF16_NP)
    r = np.arange(128)
    idmats[r, r] = 1.0
    idmats[r, 128 + r] = -BIG

    sq = (embeddings.astype(np.float64) ** 2).sum(1)
    in_maps = []
    for c in range(NCORES):
        rows = slice(c * RPC, (c + 1) * RPC)
        erolled = np.roll(embeddings, -c * RPC, axis=0)
        eT = np.ascontiguousarray(erolled.T[:, :SCOLS]).astype(BF16_NP)
        lhsT2 = np.ascontiguousarray(
            (2.0 * embeddings[rows]).T).astype(BF16_NP)
        inbf = np.concatenate([idmats, lhsT2, eT], axis=1)

        # ACT-path bias: MARGIN - sq_i - min_unit(sq_j) per (rb, unit)
        sq_rolled = np.roll(sq, -c * RPC)
        minsq_u = np.array([[sq_rolled[rb_ * 128 + c0:rb_ * 128 + c0 + w_].min()
                             for c0, w_ in UNITS]
                            for rb_ in range(NRB)])            # [NRB, NU]
        p = np.arange(128)[:, None]
        rb = np.arange(NRB)[None, :]
        sq_i = sq[c * RPC + rb * 128 + p]                      # [128, NRB]
        ab = (MARGIN - sq_i[:, :, None] - minsq_u[None, :, :])
        ab = ab.reshape(128, NRB * NU).astype(np.float32)

        clsx = (pred_cls[rows].reshape(NRB, 128, C).transpose(1, 0, 2)
                .reshape(128, NRB * C)).astype(np.float32)
        clst = (target_cls[rows].reshape(NRB, 128, C).transpose(1, 0, 2)
                .reshape(128, NRB * C)).astype(np.float32)
        boxp = (pred_boxes[rows].reshape(NRB, 128, 4).transpose(1, 2, 0)
                .reshape(128, 32)).astype(np.float32)
        boxt = (target_boxes[rows].reshape(NRB, 128, 4).transpose(1, 2, 0)
                .reshape(128, 32)).astype(np.float32)
        dn = (density_map[rows].reshape(NRB, 128).T).astype(np.float32)
        inf32 = np.ascontiguousarray(np.concatenate(
            [clsx, clst, boxp, boxt, dn, ab], axis=1))
        in_maps.append({"inbf": inbf, "inf32": inf32})
    return in_maps


def _check_certificate(results, embeddings):
    """True if some pair might have d2 <= MARGIN (then run the fallback)."""
    sq = (embeddings.astype(np.float64) ** 2).sum(1)
    p = np.arange(128)[:, None]
    rbi = np.arange(NRB)[None, :]
    NRED = NRB * NU
    for c in range(NCORES):
        red = results[c]["redout"].astype(np.float64)      # [128, 2*NRED+1]
        sq_rolled = np.roll(sq, -c * RPC)
        sq_i = sq[c * RPC + rbi * 128 + p]                 # [128, NRB]
        for rb in range(NRB):
            for u, (c0, w_) in enumerate(UNITS):
                col = rb * NU + u
                if group_kind(rb, u) == "dve":
                    mn = sq_rolled[rb * 128 + c0:rb * 128 + c0 + w_].min()
                    th = sq_i[:, rb] + mn - MARGIN
                    if (red[:, col] > th).any():
                        return True
                else:
                    if (red[:, NRED + col] > 0).any():
                        return True
    return False


def _contrastive_exact(pred_boxes, embeddings):
    """Exact numpy evaluation of the masked pairwise hinge term (fallback)."""
    pb = pred_boxes.astype(np.float64)
    e = embeddings.astype(np.float64)
    xy, wh = pb[:, :2], pb[:, 2:4] * 0.5
    a = np.concatenate([xy - wh, xy + wh], axis=1)
    area = pb[:, 2] * pb[:, 3]
    sq = (e * e).sum(1)
    total = 0.0
    CH = 512
    for i0 in range(0, N, CH):
        i1 = i0 + CH
        lt_ = np.maximum(a[i0:i1, None, :2], a[None, :, :2])
        rb_ = np.minimum(a[i0:i1, None, 2:], a[None, :, 2:])
        whp = np.clip(rb_ - lt_, 0.0, None)
        inter = whp[..., 0] * whp[..., 1]
        union = area[i0:i1, None] + area[None, :] - inter + EPS
        piou = inter / union
        d2 = sq[i0:i1, None] + sq[None, :] - 2.0 * (e[i0:i1] @ e.T)
        dist = np.sqrt(np.clip(d2, 0.0, None) + 1e-12)
        hinge = np.maximum(DELTA - dist, 0.0) ** 2
        iidx = np.arange(i0, i1)[:, None]
        mask = (iidx < np.arange(N)[None, :]) & (piou > TAU)
        total += float(hinge[mask].sum())
    return total


_PROGRAM = None


def kernel(pred_boxes, pred_cls, target_boxes, target_cls,
           embeddings, density_map, _trace=False):
    global _PROGRAM
    pred_boxes = np.asarray(pred_boxes, dtype=np.float32)
    pred_cls = np.asarray(pred_cls, dtype=np.float32)
    target_boxes = np.asarray(target_boxes, dtype=np.float32)
    target_cls = np.asarray(target_cls, dtype=np.float32)
    embeddings = np.asarray(embeddings, dtype=np.float32)
    density_map = np.asarray(density_map, dtype=np.float32)

    if _PROGRAM is None:
        _PROGRAM = build_program()
    nc = _PROGRAM
    in_maps = _prep_in_maps(pred_boxes, pred_cls, target_boxes, target_cls,
                            embeddings, density_map)
    res = run_bass_kernel_spmd(nc, in_maps, list(range(NCORES)),
                               trace=_trace)
    kernel.last_results = res

    loc_sum = 0.0
    cls_sum = 0.0
    for c in range(NCORES):
        part = res.results[c]["opart"].astype(np.float64)
        loc_sum += part[:, 0].sum()
        cls_sum += part[:, 1].sum()

    triggered = _check_certificate(res.results, embeddings)
    contrast = LAMBDA_CONTRAST * _contrastive_exact(pred_boxes, embeddings) \
        if triggered else 0.0
    kernel.last_triggered = triggered

    total = loc_sum / N + cls_sum / C + contrast
    return np.float32(total)


kernel.last_results = None
kernel.last_triggered = None


# revision 33
# speedup vs baseline: 1.0066x; 1.0066x over previous
"""Trainium2 Bass kernel for nn_DOSAConLoss (density/hardness-weighted focal CIoU
+ focal BCE + O(N^2) pairwise contrastive hinge loss).

Strategy (8 NeuronCores, shard N=8192 rows -> 1024 rows/core):
  * loss_loc  : per-row CIoU pipeline on each core's row shard (DVE+ACT, fp32).
  * loss_cls  : per-row focal BCE pipeline on each core's row shard (fp32).
  * contrast  : the hinge term max(1-dist,0)^2 is nonzero only for pairs with
                squared distance d2 < 1.  Each core screens its 1024 rows
                against ALL 8192 columns via a PE matmul producing
                2*e_i.e_j in PSUM (bf16 inputs).  The self-pair (d2=0) is
                killed on the PE itself by accumulating -BIG*I onto the
                diagonal block (each core's e^T copy is rolled by its row
                offset so the diagonal sits at a core-invariant position and
                the SPMD program can be shared).  Column groups of 2048 are
                then screened by one of two fused single-instruction paths:
                  - DVE: reduce_max -> per-row group maximum, host compares
                    against  sq_i + min_group(sq_j) - MARGIN.
                  - ACT: Relu(2dot + (MARGIN - sq_i - min_group(sq_j)))
                    with accum_out -> a sum certificate that is 0 iff no
                    pair in the group can have d2 <= MARGIN.
                If every group certifies (true for any plausible input:
                random 128-dim embeddings have min pairwise d2 ~ 92, and the
                min certified bound on this data is ~37 >> MARGIN=16 >> bf16
                error ~2), the contrastive sum is exactly 0.  Otherwise the
                host falls back to an exact numpy evaluation of the term.

Inputs are the FULL tensors from setup_inputs(); output is the scalar loss.
"""

import os
import sys

for _p in ("/opt/trn_rl_repo", "/root/.axon_site/_ro/trn_rl_repo"):
    if os.path.isdir(_p) and _p not in sys.path:
        sys.path.insert(0, _p)

from contextlib import ExitStack

import ml_dtypes
import numpy as np

import concourse.bacc as bacc
import concourse.bass as bass
import concourse.tile as tile
from concourse.tile_rust import add_dep_helper
from concourse import mybir
from concourse.bass_utils import run_bass_kernel_spmd

F32 = mybir.dt.float32
F16 = mybir.dt.float16
BF16 = mybir.dt.bfloat16
BF16_NP = ml_dtypes.bfloat16
ALU = mybir.AluOpType
AF = mybir.ActivationFunctionType

N, D, C = 8192, 128, 80
NCORES = 8
RPC = N // NCORES          # rows per core = 1024
NRB = RPC // 128           # row blocks per core = 8
CHUNK = 512                # one PSUM bank of fp32
GRP = 2048                 # column group = 4 banks
MARGIN = 16.0              # certificate slack (bf16 dot error is < ~2)
BIG = 1.0e30

# Triangular screening with per-row-block sliding windows: in a core's rolled
# frame (local col l = global j - core_base mod N), row-block rb screens
# l in [rb*128, rb*128 + WIN).  A pair at forward distance d is covered by the
# i-side block if d <= WIN-128, or by the j-side block if N-d <= WIN-128;
# min(d, N-d) <= N/2 so WIN = N/2 + 128 = 4224 covers every unordered pair.
WIN = N // 2 + 128
SCOLS = RPC - 128 + WIN    # = 5120 columns of rolled e^T actually touched
UNITS = ((0, 2048), (2048, 2048), (4096, 128))   # (win offset, width)
NU = len(UNITS)
# units flipped from the default parity assignment to balance DVE vs ACT
_FLIP = {(0, 0), (2, 2), (4, 0)}
N_WARM = 32                # PE warm-up matmuls during the DMA ramp


def group_kind(rb, u):
    """Static engine assignment for a (row-block, column-unit) tile —
    alternating so the DVE max-reduce and ACT sum-certificate paths overlap."""
    kind = "act" if (rb * NU + u) % 2 == 1 else "dve"
    if (rb, u) in _FLIP:
        kind = "act" if kind == "dve" else "dve"
    return kind

GAMMA_LOCAL = 2.5
ALPHA = 1.2
DELTA = 1.0
TAU = 0.3
LAMBDA_CONTRAST = 0.5
EPS = 1e-7


# --------------------------------------------------------------------------
# device program
# --------------------------------------------------------------------------

def build_program():
    nc = bacc.Bacc("TRN2", target_bir_lowering=False, debug=False,
                   num_devices=NCORES)

    # packed inputs: one bf16 blob (idmats | lhsT2 | eT) and one f32 blob
    # (clsx | clst | boxp | boxt | dens | abias) to minimize DMA count
    BFW = 256 + RPC + SCOLS
    FW = NRB * C * 2 + 32 + 32 + NRB + NRB * NU
    inbf = nc.dram_tensor("inbf", [128, BFW], BF16, kind="ExternalInput")
    inf32 = nc.dram_tensor("inf32", [128, FW], F32, kind="ExternalInput")

    redout = nc.dram_tensor("redout", [128, 2 * NRB * NU + 1], F32,
                            kind="ExternalOutput")
    opart = nc.dram_tensor("opart", [128, 2], F32, kind="ExternalOutput")

    with tile.TileContext(nc) as tc:
        with ExitStack() as ctx:
            consts = ctx.enter_context(tc.tile_pool(name="consts", bufs=1))
            psums = ctx.enter_context(
                tc.tile_pool(name="psums", bufs=2, space="PSUM"))
            scr = ctx.enter_context(tc.tile_pool(name="scr", bufs=3))
            work = ctx.enter_context(tc.tile_pool(name="work", bufs=1))

            bfb = consts.tile([128, BFW], BF16)
            f32b = consts.tile([128, FW], F32)
            nc.sync.dma_start(out=f32b[:], in_=inf32.ap())
            nc.sync.dma_start(out=bfb[:, :256 + RPC],
                              in_=inbf.ap()[:, :256 + RPC])
            for c0 in range(256 + RPC, BFW, 2048):
                w = min(2048, BFW - c0)
                nc.sync.dma_start(out=bfb[:, c0:c0 + w],
                                  in_=inbf.ap()[:, c0:c0 + w])
            id_s = bfb[:, 0:256]
            lhsT_s = bfb[:, 256:256 + RPC]
            eT_s = bfb[:, 256 + RPC:]
            FC_ = NRB * C
            clsx_v = f32b[:, 0:FC_]
            clst_v = f32b[:, FC_:2 * FC_]
            boxp_v = f32b[:, 2 * FC_:2 * FC_ + 32]
            boxt_v = f32b[:, 2 * FC_ + 32:2 * FC_ + 64]
            dens_v = f32b[:, 2 * FC_ + 64:2 * FC_ + 64 + NRB]
            abias_s = f32b[:, 2 * FC_ + 64 + NRB:]

            NRED = NRB * NU
            red = consts.tile([128, 2 * NRED + 1], F32)
            nc.vector.memset(red[:], -1.0)
            part = consts.tile([128, 2], F32)
            bias0 = consts.tile([128, 1], F32)
            nc.vector.memset(bias0[:], 0.0)
            bias25 = consts.tile([128, 1], F32)
            nc.vector.memset(bias25[:], 2.5)
            bias1 = consts.tile([128, 1], F32)
            nc.vector.memset(bias1[:], 1.0)

            # ---------------- focal BCE (cls) — part 1 ----------------
            FC = NRB * C
            x = clsx_v
            t = clst_v

            # softplus(-x) = ln(1 + exp(-x))   [exp/ln share one ACT table set]
            sp = work.tile([128, FC], F32)
            xn = work.tile([128, FC], F32)
            nc.vector.tensor_scalar(out=xn[:], in0=x, scalar1=-1.0,
                                    scalar2=80.0, op0=ALU.mult, op1=ALU.min)
            act_chain = []
            act_chain.append(
                nc.scalar.activation(xn[:], xn[:], AF.Exp, bias=bias0[:]))
            act_chain.append(
                nc.scalar.activation(sp[:], xn[:], AF.Ln, bias=bias1[:]))
            pr = work.tile([128, FC], F32)   # sigmoid(x)
            act_chain.append(
                nc.scalar.activation(pr[:], x, AF.Sigmoid, bias=bias0[:]))

            tx = work.tile([128, FC], F32)
            nc.vector.tensor_mul(tx[:], t, x)
            bce = work.tile([128, FC], F32)  # sp + x - t*x
            nc.vector.tensor_add(bce[:], sp[:], x)
            nc.vector.tensor_sub(bce[:], bce[:], tx)

            tp = work.tile([128, FC], F32)
            nc.vector.tensor_mul(tp[:], t, pr[:])
            w = work.tile([128, FC], F32)
            nc.vector.tensor_add(w[:], t, pr[:])
            q = work.tile([128, FC], F32)    # 1 - p_t = t + p - 2tp
            nc.vector.scalar_tensor_tensor(
                out=q[:], in0=tp[:], scalar=-2.0, in1=w[:],
                op0=ALU.mult, op1=ALU.add)
            nc.vector.tensor_scalar_max(q[:], q[:], 0.0)

            # ---------------- CIoU localization — part 1 ----------------
            NB = NRB
            bp = boxp_v.rearrange("p (c b) -> p c b", c=4)
            bt = boxt_v.rearrange("p (c b) -> p c b", c=4)
            dn = dens_v

            px, py, pw, ph = (bp[:, i, :] for i in range(4))
            tx_, ty_, tw, th = (bt[:, i, :] for i in range(4))

            loc = ctx.enter_context(tc.tile_pool(name="loc", bufs=1))

            def lt(name):
                return loc.tile([128, NB], F32, name=name)

            hw1, hh1, hw2, hh2 = lt("hw1"), lt("hh1"), lt("hw2"), lt("hh2")
            nc.vector.tensor_scalar_mul(hw1[:], pw, 0.5)
            nc.vector.tensor_scalar_mul(hh1[:], ph, 0.5)
            nc.vector.tensor_scalar_mul(hw2[:], tw, 0.5)
            nc.vector.tensor_scalar_mul(hh2[:], th, 0.5)

            l1, r1, t1, b1 = lt("l1"), lt("r1"), lt("t1"), lt("b1")
            l2, r2, t2, b2 = lt("l2"), lt("r2"), lt("t2"), lt("b2")
            nc.vector.tensor_sub(l1[:], px, hw1[:])
            nc.vector.tensor_add(r1[:], px, hw1[:])
            nc.vector.tensor_sub(t1[:], py, hh1[:])
            nc.vector.tensor_add(b1[:], py, hh1[:])
            nc.vector.tensor_sub(l2[:], tx_, hw2[:])
            nc.vector.tensor_add(r2[:], tx_, hw2[:])
            nc.vector.tensor_sub(t2[:], ty_, hh2[:])
            nc.vector.tensor_add(b2[:], ty_, hh2[:])

            # intersection / union / iou
            ltx, lty, rbx, rby = lt("ltx"), lt("lty"), lt("rbx"), lt("rby")
            nc.vector.tensor_max(ltx, l1[:], l2[:])
            nc.vector.tensor_max(lty[:], t1[:], t2[:])
            nc.vector.tensor_tensor(out=rbx, in0=r1[:], in1=r2[:], op=ALU.min)
            nc.vector.tensor_tensor(out=rby[:], in0=b1[:], in1=b2[:], op=ALU.min)
            iw, ih = lt("iw"), lt("ih")
            nc.vector.tensor_sub(iw[:], rbx, ltx)
            nc.vector.tensor_scalar_max(iw[:], iw[:], 0.0)
            nc.vector.tensor_sub(ih[:], rby[:], lty[:])
            nc.vector.tensor_scalar_max(ih[:], ih[:], 0.0)
            inter = lt("inter")
            nc.vector.tensor_mul(inter[:], iw[:], ih[:])
            area1, area2 = lt("area1"), lt("area2")
            nc.vector.tensor_mul(area1[:], pw, ph)
            nc.vector.tensor_mul(area2[:], tw, th)
            union = lt("union")
            nc.vector.tensor_add(union[:], area1[:], area2[:])
            nc.vector.scalar_tensor_tensor(
                out=union[:], in0=inter[:], scalar=-1.0, in1=union[:],
                op0=ALU.mult, op1=ALU.add)
            nc.vector.tensor_scalar_add(union[:], union[:], EPS)
            iou, runion = lt("iou"), lt("runion")
            nc.vector.reciprocal(runion[:], union[:])
            nc.vector.tensor_mul(iou[:], inter[:], runion[:])

            # enclosing box diagonal^2
            cw, chh, c2 = lt("cw"), lt("chh"), lt("c2")
            tmp = lt("tmp")
            nc.vector.tensor_max(tmp[:], r1[:], r2[:])
            nc.vector.tensor_tensor(out=cw[:], in0=l1[:], in1=l2[:], op=ALU.min)
            nc.vector.tensor_sub(cw[:], tmp[:], cw[:])
            nc.vector.tensor_max(tmp[:], b1[:], b2[:])
            nc.vector.tensor_tensor(out=chh[:], in0=t1[:], in1=t2[:], op=ALU.min)
            nc.vector.tensor_sub(chh[:], tmp[:], chh[:])
            nc.vector.tensor_mul(c2[:], cw[:], cw[:])
            nc.vector.tensor_mul(tmp[:], chh[:], chh[:])
            nc.vector.tensor_add(c2[:], c2[:], tmp[:])
            nc.vector.tensor_scalar_add(c2[:], c2[:], EPS)

            # center distance^2
            dx, dy, rho2 = lt("dx"), lt("dy"), lt("rho2")
            nc.vector.tensor_sub(dx, tx_, px)
            nc.vector.tensor_sub(dy[:], ty_, py)
            nc.vector.tensor_mul(rho2[:], dx, dx)
            nc.vector.tensor_mul(tmp[:], dy[:], dy[:])
            nc.vector.tensor_add(rho2[:], rho2[:], tmp[:])

            # v = 4/pi^2 (atan(w2/h2') - atan(w1/h1'))^2
            rat1, rat2, at1, at2, v = (lt("rat1"), lt("rat2"), lt("at1"),
                                       lt("at2"), lt("v"))
            nc.vector.tensor_scalar_add(tmp[:], ph, EPS)
            nc.vector.reciprocal(tmp[:], tmp[:])
            nc.vector.tensor_mul(rat1[:], pw, tmp[:])
            nc.vector.tensor_scalar_add(tmp[:], th, EPS)
            nc.vector.reciprocal(tmp[:], tmp[:])
            nc.vector.tensor_mul(rat2[:], tw, tmp[:])

            # HW arctan LUT covers [-pi/2, pi/2] only; for r > 1 use
            # arctan(r) = pi/2 - arctan(1/r)  (r > 0 always here).
            rr, rmin, mgt = lt("rr"), lt("rmin"), lt("mgt")
            for rat, at in ((rat1, at1), (rat2, at2)):
                nc.vector.reciprocal(rr[:], rat)
                nc.vector.tensor_tensor(out=rmin[:], in0=rat, in1=rr[:],
                                        op=ALU.min)
                act_chain.append(nc.scalar.activation(
                    at, rmin[:], AF.Arctan, bias=bias0[:]))
                nc.vector.tensor_scalar(out=mgt, in0=rat, scalar1=1.0,
                                        scalar2=None, op0=ALU.is_gt)
                # at + m*(pi/2 - 2*at)
                nc.vector.tensor_scalar(out=rr[:], in0=at, scalar1=-2.0,
                                        scalar2=float(np.pi / 2),
                                        op0=ALU.mult, op1=ALU.add)
                nc.vector.tensor_mul(mgt, mgt, rr[:])
                nc.vector.tensor_add(at, at, mgt)
            nc.vector.tensor_sub(v[:], at2[:], at1[:])
            nc.vector.tensor_mul(v[:], v[:], v[:])
            nc.vector.tensor_scalar_mul(v[:], v[:], 4.0 / (np.pi ** 2))

            # alpha = v / (v - iou + 1 + eps)
            den, alpha = lt("den"), lt("alpha")
            nc.vector.scalar_tensor_tensor(
                out=den[:], in0=iou[:], scalar=-1.0, in1=v[:],
                op0=ALU.mult, op1=ALU.add)
            nc.vector.tensor_scalar_add(den[:], den[:], 1.0 + EPS)
            nc.vector.reciprocal(den[:], den[:])
            nc.vector.tensor_mul(alpha[:], v[:], den[:])

            # ciou = iou - (rho2/c2 + v*alpha)
            ciou = lt("ciou")
            nc.vector.reciprocal(tmp[:], c2[:])
            nc.vector.tensor_mul(tmp[:], rho2[:], tmp[:])
            nc.vector.tensor_mul(alpha[:], v[:], alpha[:])
            nc.vector.tensor_add(tmp[:], tmp[:], alpha[:])
            nc.vector.tensor_sub(ciou[:], iou[:], tmp[:])

            # hardness weight sigmoid(2.5 - 5*ciou)  [sigmoid table set
            # still loaded from the arctan/sigmoid group]
            dwt, hwt = lt("dwt"), lt("hwt")
            act_chain.append(
                nc.scalar.activation(hwt, ciou[:], AF.Sigmoid,
                                     scale=-5.0, bias=bias25[:]))

            # ---------------- focal BCE (cls) — part 2 (sqrt set) ----------------
            rootq = work.tile([128, FC], F32)
            act_chain.append(
                nc.scalar.activation(rootq[:], q[:], AF.Sqrt, bias=bias0[:]))
            mod = work.tile([128, FC], F32)  # q^1.5
            nc.vector.tensor_mul(mod[:], q[:], rootq[:])
            af = work.tile([128, FC], F32)   # 0.75 - 0.5*t
            nc.vector.tensor_scalar(
                out=af[:], in0=t, scalar1=-0.5, scalar2=0.75,
                op0=ALU.mult, op1=ALU.add)
            prod = work.tile([128, FC], F32)
            nc.vector.tensor_mul(prod[:], bce[:], mod[:])
            el = work.tile([128, FC], F32)
            nc.vector.tensor_mul(el[:], prod[:], af[:])
            nc.vector.reduce_sum(out=part[:, 1:2], in_=el[:],
                                 axis=mybir.AxisListType.X)

            # ---------------- CIoU localization — part 2 ----------------
            onem, p25 = lt("onem"), lt("p25")
            nc.vector.tensor_scalar(
                out=onem[:], in0=ciou[:], scalar1=-1.0, scalar2=1.0,
                op0=ALU.mult, op1=ALU.add)
            nc.vector.tensor_scalar_max(onem[:], onem[:], 0.0)
            nc.vector.tensor_mul(p25[:], onem[:], onem[:])
            act_chain.append(
                nc.scalar.activation(tmp[:], onem[:], AF.Sqrt, bias=bias0[:]))
            nc.vector.tensor_mul(p25[:], p25[:], tmp[:])   # (1-ciou)^2.5

            saf = lt("saf")
            nc.vector.tensor_scalar_add(tmp[:], area2[:], 1e-7)
            nc.vector.reciprocal(tmp[:], tmp[:])
            nc.vector.tensor_mul(saf[:], p25[:], tmp[:])

            nc.vector.tensor_scalar(
                out=dwt[:], in0=dn, scalar1=ALPHA, scalar2=1.0,
                op0=ALU.mult, op1=ALU.add)
            nc.vector.tensor_mul(dwt[:], dwt[:], hwt[:])
            locel = lt("locel")
            nc.vector.tensor_mul(locel[:], dwt[:], saf[:])
            nc.vector.reduce_sum(out=part[:, 0:1], in_=locel[:],
                                 axis=mybir.AxisListType.X)

            # PE warm-up: ~3.5us of dummy matmuls during the input DMA so the
            # HAM clock gate releases before the real stream starts.  The
            # result is reduced into a scrap column so DCE keeps it.
            wpt = psums.tile([128, GRP], F32, name="wpt", tag="pt")
            for i in range(N_WARM):
                nc.tensor.matmul(wpt[:, 0:128], id_s[:, 0:128],
                                 id_s[:, 0:128], start=(i == 0),
                                 stop=(i == N_WARM - 1))
            nc.vector.reduce_max(out=red[:, 2 * NRED:2 * NRED + 1],
                                 in_=wpt[:, 0:128], axis=mybir.AxisListType.X)

            # ------------- pairwise screen: max(2*dot) / cert sums -------------
            for rb in range(NRB):
                lhs_slice = lhsT_s[:, rb * 128:(rb + 1) * 128]
                base = rb * 128            # window start; diag block at offset 0
                for u, (c0, w) in enumerate(UNITS):
                    pt = psums.tile([128, GRP], F32, name="pt", tag="pt")
                    for cc in range((w + CHUNK - 1) // CHUNK):
                        cw = min(CHUNK, w - cc * CHUNK)
                        nc.tensor.matmul(
                            pt[:, cc * CHUNK:cc * CHUNK + cw], lhs_slice,
                            eT_s[:, base + c0 + cc * CHUNK:
                                 base + c0 + cc * CHUNK + cw],
                            start=True, stop=not (u == 0 and cc == 0))
                    if u == 0:
                        # kill the self-pair: accumulate -BIG*I onto the
                        # diagonal 128x128 block (window offset 0)
                        nc.tensor.matmul(
                            pt[:, 0:128],
                            id_s[:, 128:256], id_s[:, 0:128],
                            start=False, stop=True)
                    col = rb * NU + u
                    if group_kind(rb, u) == "dve":
                        nc.vector.reduce_max(
                            out=red[:, col:col + 1], in_=pt[:, :w],
                            axis=mybir.AxisListType.X)
                    else:
                        o16 = scr.tile([128, GRP], F16, name="o16",
                                       tag="act_scratch")
                        nc.scalar.activation(
                            o16[:, :w], pt[:, :w], AF.Relu,
                            bias=abias_s[:, col:col + 1],
                            scale=1.0,
                            accum_out=red[:, NRED + col:NRED + col + 1])

            nc.sync.dma_start(out=redout.ap(), in_=red[:])

            nc.sync.dma_start(out=opart.ap(), in_=part[:])

            # pin the transcendental order so the ACT table sets load at most
            # once each: [exp ln] [sigmoid arctan arctan sigmoid] [sqrt sqrt]
            for a, b in zip(act_chain[1:], act_chain[:-1]):
                add_dep_helper(a.ins, b.ins, sync=False,
                               reason="group ACT calls by table set")

    nc.compile()
    return nc


# --------------------------------------------------------------------------
# host-side prep / gather
# --------------------------------------------------------------------------

def _prep_in_maps(pred_boxes, pred_cls, target_boxes, target_cls,
                  embeddings, density_map):
    idmats = np.zeros((128, 256), BF16_NP)
    r = np.arange(128)
    idmats[r, r] = 1.0
    idmats[r, 128 + r] = -BIG

    sq = (embeddings.astype(np.float64) ** 2).sum(1)
    in_maps = []
    for c in range(NCORES):
        rows = slice(c * RPC, (c + 1) * RPC)
        erolled = np.roll(embeddings, -c * RPC, axis=0)
        eT = np.ascontiguousarray(erolled.T[:, :SCOLS]).astype(BF16_NP)
        lhsT2 = np.ascontiguousarray(
            (2.0 * embeddings[rows]).T).astype(BF16_NP)
        inbf = np.concatenate([idmats, lhsT2, eT], axis=1)

        # ACT-path bias: MARGIN - sq_i - min_unit(sq_j) per (rb, unit)
        sq_rolled = np.roll(sq, -c * RPC)
        minsq_u = np.array([[sq_rolled[rb_ * 128 + c0:rb_ * 128 + c0 + w_].min()
                             for c0, w_ in UNITS]
                            for rb_ in range(NRB)])            # [NRB, NU]
        p = np.arange(128)[:, None]
        rb = np.arange(NRB)[None, :]
        sq_i = sq[c * RPC + rb * 128 + p]                      # [128, NRB]
        ab = (MARGIN - sq_i[:, :, None] - minsq_u[None, :, :])
        ab = ab.reshape(128, NRB * NU).astype(np.float32)

        clsx = (pred_cls[rows].reshape(NRB, 128, C).transpose(1, 0, 2)
                .reshape(128, NRB * C)).astype(np.float32)
        clst = (target_cls[rows].reshape(NRB, 128, C).transpose(1, 0, 2)
                .reshape(128, NRB * C)).astype(np.float32)
        boxp = (pred_boxes[rows].reshape(NRB, 128, 4).transpose(1, 2, 0)
                .reshape(128, 32)).astype(np.float32)
        boxt = (target_boxes[rows].reshape(NRB, 128, 4).transpose(1, 2, 0)
                .reshape(128, 32)).astype(np.float32)
        dn = (density_map[rows].reshape(NRB, 128).T).astype(np.float32)
        inf32 = np.ascontiguousarray(np.concatenate(
            [clsx, clst, boxp, boxt, dn, ab], axis=1))
        in_maps.append({"inbf": inbf, "inf32": inf32})
    return in_maps


def _check_certificate(results, embeddings):
    """True if some pair might have d2 <= MARGIN (then run the fallback)."""
    sq = (embeddings.astype(np.float64) ** 2).sum(1)
    p = np.arange(128)[:, None]
    rbi = np.arange(NRB)[None, :]
    NRED = NRB * NU
    for c in range(NCORES):
        red = results[c]["redout"].astype(np.float64)      # [128, 2*NRED+1]
        sq_rolled = np.roll(sq, -c * RPC)
        sq_i = sq[c * RPC + rbi * 128 + p]                 # [128, NRB]
        for rb in range(NRB):
            for u, (c0, w_) in enumerate(UNITS):
                col = rb * NU + u
                if group_kind(rb, u) == "dve":
                    mn = sq_rolled[rb * 128 + c0:rb * 128 + c0 + w_].min()
                    th = sq_i[:, rb] + mn - MARGIN
                    if (red[:, col] > th).any():
                        return True
                else:
                    if (red[:, NRED + col] > 0).any():
                        return True
    return False


def _contrastive_exact(pred_boxes, embeddings):
    """Exact numpy evaluation of the masked pairwise hinge term (fallback)."""
    pb = pred_boxes.astype(np.float64)
    e = embeddings.astype(np.float64)
    xy, wh = pb[:, :2], pb[:, 2:4] * 0.5
    a = np.concatenate([xy - wh, xy + wh], axis=1)
    area = pb[:, 2] * pb[:, 3]
    sq = (e * e).sum(1)
    total = 0.0
    CH = 512
    for i0 in range(0, N, CH):
        i1 = i0 + CH
        lt_ = np.maximum(a[i0:i1, None, :2], a[None, :, :2])
        rb_ = np.minimum(a[i0:i1, None, 2:], a[None, :, 2:])
        whp = np.clip(rb_ - lt_, 0.0, None)
        inter = whp[..., 0] * whp[..., 1]
        union = area[i0:i1, None] + area[None, :] - inter + EPS
        piou = inter / union
        d2 = sq[i0:i1, None] + sq[None, :] - 2.0 * (e[i0:i1] @ e.T)
        dist = np.sqrt(np.clip(d2, 0.0, None) + 1e-12)
        hinge = np.maximum(DELTA - dist, 0.0) ** 2
        iidx = np.arange(i0, i1)[:, None]
        mask = (iidx < np.arange(N)[None, :]) & (piou > TAU)
        total += float(hinge[mask].sum())
    return total


_PROGRAM = None


def kernel(pred_boxes, pred_cls, target_boxes, target_cls,
           embeddings, density_map, _trace=False):
    global _PROGRAM
    pred_boxes = np.asarray(pred_boxes, dtype=np.float32)
    pred_cls = np.asarray(pred_cls, dtype=np.float32)
    target_boxes = np.asarray(target_boxes, dtype=np.float32)
    target_cls = np.asarray(target_cls, dtype=np.float32)
    embeddings = np.asarray(embeddings, dtype=np.float32)
    density_map = np.asarray(density_map, dtype=np.float32)

    if _PROGRAM is None:
        _PROGRAM = build_program()
    nc = _PROGRAM
    in_maps = _prep_in_maps(pred_boxes, pred_cls, target_boxes, target_cls,
                            embeddings, density_map)
    res = run_bass_kernel_spmd(nc, in_maps, list(range(NCORES)),
                               trace=_trace)
    kernel.last_results = res

    loc_sum = 0.0
    cls_sum = 0.0
    for c in range(NCORES):
        part = res.results[c]["opart"].astype(np.float64)
        loc_sum += part[:, 0].sum()
        cls_sum += part[:, 1].sum()

    triggered = _check_certificate(res.results, embeddings)
    contrast = LAMBDA_CONTRAST * _contrastive_exact(pred_boxes, embeddings) \
        if triggered else 0.0
    kernel.last_triggered = triggered

    total = loc_sum / N + cls_sum / C + contrast
    return np.float32(total)


kernel.last_results = None
kernel.last_triggered = None
